# revision 70
# baseline (speedup 1.0000x reference)
"""NT-Xent loss on 8 Trainium2 NeuronCores — fp8 + symmetric 5/8 scheme.

Math: z = concat(z_i, z_j) [8192, 256]; E = exp(2*cos_sim(z)) is
symmetric, so row sums = column sums. Blocked over 8 superblocks of
1024 rows, core c owns rows [1024c, 1024c+1024) and computes sim
against column superblocks d = 0..4 (its own + the next four, mod 8).
d in {1,2,3} blocks are mirrored via COLUMN sums (row sums of the
uncomputed transpose block). The d=4 block forms a transpose-dual pair
across cores (c, c+4): rows m>=4 compute only cols [512:1024) and the
missing cols [0:512) mass is mirrored from the pair core's colsum over
its m0..3 rows of cols [512:1024) (d4-mirror; only the two diagonal
512-squares are double-computed, counted once each). The host
assembles denom = sum - e^2, takes ln in fp64, and averages — O(2N).

Per core (identical SPMD program on column-rotated inputs):
  - Host normalizes rows in fp64, scales by 16, quantizes to fp8 e4m3,
    ships chunk-major znt[p, 2c+h, j] = q[1024c+j, 128h+p] so every
    1024-col superblock is 2 KiB contiguous per partition on both DMA
    sides. Two DMA rings in baseline order (sync: c0,c2,c4; gpsimd:
    c1,id,c3); ones comes from a gpsimd memset (a [P,1] DMA is 128
    tiny descriptors that stall a ring). A third ring (scalar HWDGE)
    and finer chunk splits measured WORSE: all rings share one AXI
    port (~200 GB/s) and late-deadline chunks steal bandwidth from
    the critical first window's data.
  - Sim matmuls: fp8 DoubleRow, one matmul per [128, 512] output
    covering the full K=256 contraction. PSUM = 256*sim.
  - d0-triangle skip: for row-chunks m>=4, cols [0:512) of the self
    superblock are strictly below the diagonal — skipped, mass comes
    back via the d0-mirror colsum (rows 0:512 of cols [512:1024)).
  - ACT exps [128, 2048] PSUM windows (scale 1/128) with fused row-sum
    accumulation, writing exp as bf16 to SBUF (esb) for the colsum
    path. FIVE of the 16 q-windows are offloaded to DVE via int16
    Schraudolph: int16(A/2^16*x + B/2^16) IS the bf16 bit pattern of
    exp(x), written straight into esb via .bitcast(I16); the rowsum is
    a flat bf16 copy-reduce (CACHE_REDUCE has no 2x uop; fold trees
    measured worse under DVE<->gpsimd SBUF contention).
  - A d=4 window rides behind the window FOLLOWING each Sch window:
    the 2-slot PSUM rotation goes sch(A) -> next-q(B) -> ride(A, freed
    by pass1) -> q+2(B). Riding directly behind the Sch window puts
    the ride in slot B and costs ~2 us of ACT idle per Sch cycle
    (pass1 holds slot A ~2.3 us). Trailing d4 windows m5+m6 share one
    PSUM tile (both 512-col after d4-mirror), m7 takes the next slot.
  - Colsum trees (bf16 pair-adds over the 8 m-chunks, emitted
    incrementally as chunks land): q0 + d0 trees on GpSimd (deep DVE
    queues around the Sch offloads cost more than gpsimd<->DVE SBUF
    contention), q1 + d4sum trees on DVE (2x, tight tail deadline).
    Tail colsum ones-matmuls grouped by READINESS: (0,1,6,7) = esum0/
    d0sum/d4sum are ready mid-stream; (2,3,4,5) = esum1 waits only on
    the q1 tree (~2.4 us after the last window).
  - Outputs: rowpos [128, 16] (rowsum partials | exp'd positives) and
    colsums [1, 4096] — ONE DMA each; every DMA_DIRECT2D costs ~0.6 us
    of engine issue time, so per-block output DMAs wreck the tail.

Measured pitfalls baked in (each cost a hardware run to learn):
  - DVE tensor_reduce AND tensor_scalar CACHE_REDUCE have no 2x uop.
  - gpsimd partition_all_reduce is ~5x slower than its cost model
    (use ones-matmuls); gpsimd tensor_tensor bf16 adds DO match the
    model (~2.1 us per [128,1024]) and gpsimd has no PSUM port; its
    scalar_tensor_tensor with accum_out does not compile (neuronxcc).
  - DVE<->gpsimd SBUF contention stretches overlapping ops 1.5-2.5x
    (ACT never stretches). Fewer, larger ops win.
  - fp8 DoubleRow streams ~0.5-0.6 ns/moving-row; ACT ~0.96 ns/col
    with no 2x for any dtype; ACT accum flush = 284 ns per op.
  - Framework preamble ~7 us, DMA doorbell ~1.6 us after issue, final
    barrier+drain ~4.5 us: ~13 us of fixed overhead.
  - Window splitting and ride-spreading LOSE (3 PSUM consumers per
    2-slot turn starves the rotation: +6..+9 us). q1-before-q0 LOSES
    (window 0 then needs 768 KiB of DMA instead of 512 KiB).
  - Run-to-run jitter is +-2-3 us, and the device occasionally runs
    whole kernels at ~9% lower clock (check: 2048-col ACTIVATE =
    1967 ns at full clock) — verify clock before comparing runs.

fp8 error budget: e4m3 sigma~3.6%/elem -> sim noise sigma~3e-3 which
averages out in row sums and the 8192-row mean; Schraudolph adds ~3%
per-element on 5/16 of the denominator mass -> ~1e-4 loss error.
The q1 tree stops at the half-sums t1_1_0/t1_1_4; the tail colsum
matmuls accumulate the two halves per 512-col PSUM bank (a matmul may
not cross a bank boundary), so the tail chain is only
(1,7)exp -> t1_0_6 -> t1_1_4 -> 4 stop-matmuls -> copies -> one DMA.
Verified: 61356-62688 ns HW (baseline 63154 same-day), rel err 1.2e-4.
"""

import os
import sys

sys.path.insert(0, "/opt/trn_rl_repo")
os.environ.setdefault("MYCRO_LOCAL_CACHE", "1")

import numpy as np

import concourse.bass as bass
import concourse.mybir as mybir
from concourse import bacc, tile
from concourse.bass_utils import run_bass_kernel_spmd

F32 = mybir.dt.float32
BF16 = mybir.dt.bfloat16
FP8 = mybir.dt.float8e4
I32 = mybir.dt.int32
I16 = mybir.dt.int16
AF = mybir.ActivationFunctionType
ALU = mybir.AluOpType
DROW = mybir.MatmulPerfMode.DoubleRow

N_CORES = 8
TWO_N = 8192
D = 256
P = 128
ROWS_PER_CORE = TWO_N // N_CORES  # 1024
M_CHUNKS = ROWS_PER_CORE // P     # 8 local row chunks
NCOL = 512                        # matmul free dim (one PSUM bank)
QCOL = 2048                       # ACT window for q=0,1
N_CHUNKS = 5                      # 1024-col superblocks held per core
LCOLS = 5 * ROWS_PER_CORE         # 5120 local cols (superblocks d=0..4)
CS_LO, CS_HI = 1024, 4096         # colsum region (d=1..3)
D0CS = 512                        # d0 below-diag mirror segment (cols
                                  # 512:1024 summed over rows 0:512)
D4CS = 512                        # d4-mirror segment (cols 4608:5120
                                  # summed over rows 0:512)
CS_TOT = CS_HI - CS_LO + D0CS + D4CS   # 4096 colsum outputs
POS_Q0 = 4096                     # start of the d=4 window
N_WIN = 3                         # rowsum slots per m: q0, q1, d4
TEMP_SCALE = 2.0
QSCALE = 16.0                     # host fp8 quant scale; psum = 256*sim
ACT_SCALE = TEMP_SCALE / (QSCALE * QSCALE)   # exp(psum/128)
POS_SCALE = TEMP_SCALE / (QSCALE * QSCALE)

# Schraudolph: exp(x) ~ bits_as_f32(int32(2^23/ln2 * x + 127*2^23 - C))
SCH_A = ACT_SCALE * (1 << 23) / float(np.log(2.0))   # applied to psum
SCH_B = float(127 * (1 << 23) - 366393)
# int16 variant: the int16 of (SCH_A/2^16 * x + SCH_B/2^16) is the upper
# half of the f32 bit pattern = the bf16 of exp(x), written directly
SCH_A16 = SCH_A / 65536.0
SCH_B16 = SCH_B / 65536.0
# windows (q, m) whose exp+rowsum runs on DVE instead of ACT
SCH_WINDOWS = {(0, 2), (0, 5), (1, 0), (1, 3), (1, 6)}
# Sch windows whose pass2 rowsum-reduce runs on ACT instead of DVE.
# Tried {(1,0)}: +2.3 us — DVE is ~92% busy mid-stream vs ACT ~78%,
# but the 2.25 us ACT reduce overflows ACT's ~1.3 us bubbles and
# delays the window chain. Keep empty.
SCH_ACT_REDUCE = set()

_NC_CACHE = {}


def _build_nc():
    nc = bacc.Bacc(
        "TRN2",
        target_bir_lowering=False,
        debug=False,
        enable_asserts=False,
        num_devices=N_CORES,
    )
    # chunk-major layout: znt[p, 2*c + h, j] = q[1024*c + j, 128*h + p]
    # so each 1024-col superblock is 2 KiB contiguous per partition on
    # both the DRAM and SBUF side (fast DMA lines)
    znt = nc.dram_tensor("znt", [P, 2 * N_CHUNKS, 1024], FP8,
                         kind="ExternalInput")
    ident = nc.dram_tensor("ident", [P, P], BF16, kind="ExternalInput")
    rowpos_d = nc.dram_tensor("rowpos", [P, 2 * M_CHUNKS], F32,
                              kind="ExternalOutput")
    colsums_d = nc.dram_tensor("colsums", [1, CS_TOT], F32,
                               kind="ExternalOutput")

    with tile.TileContext(nc) as tc:
        with (
            tc.tile_pool(name="big", bufs=1) as big,
            tc.tile_pool(name="esbp", bufs=1) as esbp,
            tc.tile_pool(name="work", bufs=2) as work,
        ):
            znt_sb = big.tile([P, 2 * N_CHUNKS, 1024], FP8)

            def zv(c0, width=NCOL):
                """[128, 2, width] matmul-operand view of global cols
                [c0, c0+width) — must stay inside one 1024-col chunk."""
                c, j = divmod(c0, 1024)
                assert j + width <= 1024
                return znt_sb[:, 2 * c:2 * c + 2, j:j + width]

            # two DMA rings, exactly the baseline's queue assignment and
            # order (a third ring and finer splits measured WORSE: the
            # rings share one AXI port and competition from late-deadline
            # chunks slows the critical first chunks). Chunk-major layout
            # still gives 2 KiB contiguous lines on both sides.
            # c1 is split across both rings so window (0,0)'s full data
            # (c0 + c1) lands ~1 us sooner; c1b heads the gpsimd ring
            nc.sync.dma_start(znt_sb[:, 0:2, :], znt[:, 0:2, :])
            nc.gpsimd.dma_start(znt_sb[:, 2:4, 512:1024],
                                znt[:, 2:4, 512:1024])
            id_sb = big.tile([P, P], BF16)
            ones_sb = big.tile([P, 1], BF16)
            nc.gpsimd.memset(ones_sb[:], 1.0)
            nc.sync.dma_start(znt_sb[:, 2:4, 0:512], znt[:, 2:4, 0:512])
            nc.gpsimd.dma_start(id_sb[:], ident[:])
            nc.sync.dma_start(znt_sb[:, 4:6, :], znt[:, 4:6, :])
            nc.gpsimd.dma_start(znt_sb[:, 6:8, :], znt[:, 6:8, :])
            nc.sync.dma_start(znt_sb[:, 8:10, :], znt[:, 8:10, :])

            # preload the Exp activation table while DMAs stream; read a
            # memset scratch so the preload has no DMA dependency
            tblin = big.tile([P, 1], F32)
            nc.gpsimd.memset(tblin[:], 0.0)
            tbl = big.tile([P, 1], F32)
            nc.scalar.activation(tbl[:], tblin[:], AF.Exp)

            sums = big.tile([P, M_CHUNKS * N_WIN], F32)
            # rowpos[:, 0:8] = rowsum partials (stot), [:, 8:16] = posv;
            # combined so one DMA issue flushes both
            rowpos = big.tile([P, 2 * M_CHUNKS], F32)
            pos = rowpos[:, M_CHUNKS:2 * M_CHUNKS]
            # exp windows for q=0,1 (cols 0:4096), bf16, m-major
            esb = [esbp.tile([P, M_CHUNKS, QCOL], BF16, tag=f"esb{q}",
                             name=f"esb{q}")
                   for q in range(2)]
            esum = [esbp.tile([P, 1024], BF16, tag="esum0", name="esum0"),
                    esbp.tile([P, QCOL], BF16, tag="esum1", name="esum1")]
            colsum_sb = big.tile([1, CS_TOT], F32)
            d4buf = esbp.tile([P, M_CHUNKS, 1024], BF16, tag="d4b",
                              name="d4buf")

            tree_t = {}

            def tree_step(dst, src_m, sl, width, q, m):
                """Emit the bf16 adds that become ready once chunk m of
                src_m[:, :, sl] is written. q0's tree finishes with a
                full m-sum in dst (gpsimd: deep DVE queues around the
                Schraudolph offloads cost more than the gpsimd<->DVE
                SBUF contention they avoid). q1's tree STOPS at the two
                half-sums t1_1_0/t1_1_4 — the colsum matmuls accumulate
                them in PSUM, cutting the final 1.2 us add off the
                tail-gating chain. (Tried q1's early half on gpsimd
                too — its slow 2048-wide adds stretch DVE via SBUF
                contention more than they relieve its queue: +1.5 us.)"""
                eng = nc.gpsimd if q == 0 else nc.vector
                def t(tag):
                    if tag not in tree_t:
                        tree_t[tag] = work.tile([P, width], BF16, tag=tag,
                                                bufs=1, name=tag)
                    return tree_t[tag]
                if m % 2 == 1:
                    a = t(f"t{q}_0_{m - 1}")
                    eng.tensor_tensor(a[:], src_m[:, m - 1, sl],
                                      src_m[:, m, sl], ALU.add)
                if m == 3:
                    b = t(f"t{q}_1_0")
                    eng.tensor_tensor(b[:], t(f"t{q}_0_0")[:],
                                      t(f"t{q}_0_2")[:], ALU.add)
                if m == 7:
                    b = t(f"t{q}_1_4")
                    eng.tensor_tensor(b[:], t(f"t{q}_0_4")[:],
                                      t(f"t{q}_0_6")[:], ALU.add)
                    if q == 0:
                        eng.tensor_tensor(dst, t(f"t{q}_1_0")[:],
                                          b[:], ALU.add)

            with tc.tile_pool(name="psum", bufs=2, space="PSUM") as psum_pool:
                def d4_window(m, ptf=None, base=0):
                    """d=4 window interleaved into the q stream: MMs into
                    a half-used sim-tag tile, exp to SBUF d4buf, post-exp
                    diagonal extract (2x bf16 STT, no PSUM interlock).
                    d4-mirror: the (c, c+4) block pair is transpose-dual,
                    so rows m>=4 compute only cols [512:1024) and recover
                    cols [0:512) mass from the PAIR core's colsum over its
                    m0..3 rows of cols [512:1024) (the d4sum tree below);
                    only the two diagonal 512-squares are double-computed,
                    and those enter rowsums once each (no double count).
                    Pass ptf/base to pack two 512-col windows in one tile
                    (the trailing windows otherwise serialize on slot
                    rotation)."""
                    if ptf is None:
                        ptf = psum_pool.tile([P, QCOL], F32, tag="sim",
                                             name="ptd4")
                    pt = ptf[:, base:base + 1024]
                    lhsT = zv(m * P, P)
                    lo4 = 0 if m < 4 else NCOL
                    for nn in range(lo4 // NCOL, 2):
                        col = POS_Q0 + nn * NCOL
                        nc.tensor.matmul(
                            pt[:, nn * NCOL:(nn + 1) * NCOL],
                            lhsT,
                            zv(col),
                            start=True, stop=True, perf_mode=DROW)
                    col_ix = m * N_WIN + 2
                    nc.scalar.activation(
                        d4buf[:, m, lo4:], pt[:, lo4:], AF.Exp,
                        scale=ACT_SCALE,
                        accum_out=sums[:, col_ix:col_ix + 1])
                    off = m * P
                    scr = work.tile([P, P], BF16, tag="extr")
                    nc.vector.scalar_tensor_tensor(
                        out=scr[:], in0=d4buf[:, m, off:off + P],
                        scalar=1.0, in1=id_sb[:],
                        op0=ALU.mult, op1=ALU.mult,
                        accum_out=pos[:, m:m + 1])
                    if m == 3:
                        # d4-mirror tree on gpsimd: 3 NARROW 512-col
                        # adds (the only gpsimd-profitable width), ~8 us
                        # of slack before the block-7 colsum matmul, and
                        # gpsimd is idle here (q0 tree done) — frees
                        # ~1 us of the DVE pacer
                        ta = work.tile([P, NCOL], BF16, tag="d4a",
                                       bufs=1, name="d4a")
                        nc.gpsimd.tensor_tensor(
                            ta[:], d4buf[:, 0, 512:1024],
                            d4buf[:, 1, 512:1024], ALU.add)
                        tb = work.tile([P, NCOL], BF16, tag="d4b2",
                                       bufs=1, name="d4b2")
                        nc.gpsimd.tensor_tensor(
                            tb[:], d4buf[:, 2, 512:1024],
                            d4buf[:, 3, 512:1024], ALU.add)
                        td = work.tile([P, NCOL], BF16, tag="d4s",
                                       bufs=1, name="d4sum")
                        nc.gpsimd.tensor_tensor(td[:], ta[:], tb[:],
                                                ALU.add)
                        tree_t["d4sum"] = td

                # q0 phase first (q1-first measured far worse: window 0
                # then needs c0+c2+c3 = 768 KiB before it can finish vs
                # 512 KiB for q0-first). A d4 window rides behind the
                # window FOLLOWING each DVE-offloaded one: the 2-slot
                # rotation then goes sch(A) -> next-q(B) -> ride(A,
                # freed by pass1) -> q+2(B). Riding directly behind the
                # Sch window put the ride in slot B and made the next
                # q-window wait out pass1's ~2.3 us hold of slot A
                # (~2 us ACT gap per Sch cycle).
                d4_done = 0
                ride_pending = False
                for q in (0, 1):        # esb cols: q0 [0:2048), q1 [2048:4096)
                    for m in range(M_CHUNKS):
                        pt = psum_pool.tile([P, QCOL], F32, tag="sim")
                        lhsT = zv(m * P, P)
                        # cols [0:512) of the self-superblock are strictly
                        # below the diagonal for rows m>=4: skip them and
                        # recover the row-sum mass from the d0-mirror
                        # colsum over rows m0..3 of cols [512:1024)
                        lo = NCOL if (q == 0 and m >= 4) else 0
                        for nn in range(lo // NCOL, QCOL // NCOL):
                            col = q * QCOL + nn * NCOL
                            nc.tensor.matmul(
                                pt[:, nn * NCOL:(nn + 1) * NCOL],
                                lhsT,
                                zv(col),
                                start=True, stop=True, perf_mode=DROW)
                        col_ix = m * N_WIN + q
                        if (q, m) in SCH_WINDOWS:
                            # Schraudolph exp on DVE, single pass: int16 of
                            # (A16*x + B16) IS the bf16 of exp(x), written
                            # straight into esb (frees the PSUM bank);
                            # rowsum via a flat bf16 copy-reduce (folds and
                            # multi-op variants measured WORSE under
                            # DVE<->gpsimd SBUF contention)
                            nc.vector.tensor_scalar(
                                esb[q][:, m, lo:].bitcast(I16),
                                pt[:, lo:], SCH_A16, SCH_B16,
                                ALU.mult, ALU.add)
                            scr = work.tile([P, QCOL], BF16, tag="sch",
                                            name="schscr")
                            # NOTE: tensor_tensor_reduce(max(e,e)) as a
                            # 2x-mode shot passed CoreSim but crashed at
                            # HW execution (INTERNAL runtime error) —
                            # stay on the 1x tensor_scalar cache-reduce
                            nc.vector.tensor_scalar(
                                scr[:, lo:], esb[q][:, m, lo:],
                                1.0, 0.0, ALU.mult, ALU.add,
                                accum_out=sums[:, col_ix:col_ix + 1])
                        else:
                            nc.scalar.activation(
                                esb[q][:, m, lo:], pt[:, lo:], AF.Exp,
                                scale=ACT_SCALE,
                                accum_out=sums[:, col_ix:col_ix + 1])
                        if (q, m) == (0, 3):
                            # d0-mirror tree: sum rows m0..3 of cols
                            # [512:1024) (bf16, 2x mode)
                            ta = work.tile([P, D0CS], BF16, tag="d0a",
                                           bufs=1, name="d0a")
                            nc.gpsimd.tensor_tensor(
                                ta[:], esb[0][:, 0, 512:1024],
                                esb[0][:, 1, 512:1024], ALU.add)
                            tb = work.tile([P, D0CS], BF16, tag="d0b",
                                           bufs=1, name="d0b")
                            nc.gpsimd.tensor_tensor(
                                tb[:], esb[0][:, 2, 512:1024],
                                esb[0][:, 3, 512:1024], ALU.add)
                            td = work.tile([P, D0CS], BF16, tag="d0s",
                                           bufs=1, name="d0sum")
                            nc.gpsimd.tensor_tensor(td[:], ta[:], tb[:],
                                                    ALU.add)
                            tree_t["d0sum"] = td
                        # colsum m-tree, incrementally as chunks land
                        if q == 0:
                            tree_step(esum[0][:], esb[0], slice(1024, 2048),
                                      1024, 0, m)
                        else:
                            tree_step(esum[1][:], esb[1], slice(0, QCOL),
                                      QCOL, 1, m)
                        if ride_pending:
                            d4_window(d4_done)
                            d4_done += 1
                            ride_pending = False
                        if (q, m) in SCH_WINDOWS and d4_done < 5:
                            ride_pending = True
                # trailing d4 windows: m5+m6 share one PSUM tile (both
                # are 512-col), m7 takes the next rotation slot
                assert d4_done == 5
                pair = psum_pool.tile([P, QCOL], F32, tag="sim",
                                      name="ptd4pair")
                d4_window(5, ptf=pair, base=0)
                d4_window(6, ptf=pair, base=1024)
                d4_window(7)

                # rowsum partials + posv go out as soon as the last
                # window's accum lands (before the colsum tail); one
                # combined DMA (each DMA_DIRECT2D costs ~0.6 us of
                # sync-queue issue time)
                nc.vector.tensor_reduce(
                    rowpos[:, 0:M_CHUNKS],
                    sums[:].rearrange("p (m q) -> p m q", q=N_WIN),
                    axis=mybir.AxisListType.X,
                    op=ALU.add,
                )
                nc.sync.dma_start(rowpos_d[:], rowpos[:])

                # colsum matmuls ride the tail of the sim ring, grouped
                # by READINESS. Group A (esum0 + d0sum + d4sum, blocks
                # 0,1,6,7) is complete by mid-stream and overlaps the
                # last windows. Group B (esum1 region, blocks 2..5) is
                # TWO wide [1, 2048] ones-matmuls accumulating the q1
                # half-tree sums in PSUM: the t1_1_0 matmul is ready
                # mid-stream, so the tail chain is only
                # (1,7)exp -> t1_0_6 -> t1_1_4 -> one matmul -> copies.
                ptcsA = psum_pool.tile([P, QCOL], F32, tag="sim",
                                       name="ptcsA")
                for i, b in enumerate((0, 1, 6, 7)):
                    if b == 6:
                        src = tree_t["d0sum"][:]
                    elif b == 7:
                        src = tree_t["d4sum"][:]
                    else:
                        src = esum[0][:, b * NCOL:(b + 1) * NCOL]
                    nc.tensor.matmul(ptcsA[0:1, i * NCOL:(i + 1) * NCOL],
                                     ones_sb[:], src,
                                     start=True, stop=True)
                for half in range(2):
                    b0 = (0, 6)[half]
                    psl = ptcsA[0:1, 2 * half * NCOL:2 * (half + 1) * NCOL]
                    dst = colsum_sb[:, b0 * NCOL:(b0 + 2) * NCOL]
                    if half == 0:
                        nc.vector.tensor_copy(dst, psl)
                    else:
                        nc.scalar.copy(dst, psl)

                ptcsB = psum_pool.tile([P, QCOL], F32, tag="sim",
                                       name="ptcsB")
                # per-bank accumulating pairs (a matmul may not cross a
                # PSUM bank): the 4 t1_1_0 matmuls are ready mid-stream,
                # only the 4 t1_1_4 ones ride the tail chain
                for i in range(4):
                    sl = slice(i * NCOL, (i + 1) * NCOL)
                    nc.tensor.matmul(ptcsB[0:1, sl], ones_sb[:],
                                     tree_t["t1_1_0"][:, sl],
                                     start=True, stop=False)
                for i in range(4):
                    sl = slice(i * NCOL, (i + 1) * NCOL)
                    nc.tensor.matmul(ptcsB[0:1, sl], ones_sb[:],
                                     tree_t["t1_1_4"][:, sl],
                                     start=False, stop=True)
                nc.vector.tensor_copy(colsum_sb[:, 1024:2048],
                                      ptcsB[0:1, 0:1024])
                nc.scalar.copy(colsum_sb[:, 2048:3072],
                               ptcsB[0:1, 1024:2048])
                nc.sync.dma_start(colsums_d[:], colsum_sb[:])

    nc.compile()
    return nc


def _get_nc():
    if "nc" not in _NC_CACHE:
        _NC_CACHE["nc"] = _build_nc()
    return _NC_CACHE["nc"]


def _prepare_in_maps(z_i, z_j):
    import ml_dtypes

    z = np.concatenate(
        [np.asarray(z_i, np.float64), np.asarray(z_j, np.float64)], axis=0
    )
    zn = z / np.linalg.norm(z, axis=1, keepdims=True)
    q = (zn * QSCALE).astype(np.float32).astype(ml_dtypes.float8_e4m3)
    # znt[p, h, j] = q[j, h*128 + p]
    znt = np.ascontiguousarray(q.T.reshape(2, P, TWO_N).transpose(1, 0, 2))
    ident = np.eye(P, dtype=ml_dtypes.bfloat16)
    in_maps = []
    for c in range(N_CORES):
        zc = np.roll(znt, -ROWS_PER_CORE * c, axis=2)[:, :, :LCOLS]
        # chunk-major: znt10[p, 2*c + h, j] = zc[p, h, 1024*c + j]
        zc10 = zc.reshape(P, 2, N_CHUNKS, 1024).transpose(0, 2, 1, 3)
        zc10 = np.ascontiguousarray(zc10.reshape(P, 2 * N_CHUNKS, 1024))
        in_maps.append(
            {"znt": zc10, "ident": ident})
    return in_maps


def _combine(results):
    """Assemble the loss from per-core rowsum/colsum/pos partials."""
    total = np.zeros(TWO_N, dtype=np.float64)
    posg = np.zeros(TWO_N, dtype=np.float64)
    for c in range(N_CORES):
        r0 = c * ROWS_PER_CORE
        rp = np.asarray(results[c]["rowpos"], np.float64)   # [128, 16]
        rs, pv = rp[:, :M_CHUNKS], rp[:, M_CHUNKS:]
        for m in range(M_CHUNKS):
            gsl = slice(r0 + m * P, r0 + (m + 1) * P)
            total[gsl] += rs[:, m]
            posg[gsl] = pv[:, m]
        cs = np.asarray(results[c]["colsums"], np.float64).ravel()  # [4096]
        n_main = CS_HI - CS_LO
        gidx = (r0 + CS_LO + np.arange(n_main)) % TWO_N
        np.add.at(total, gidx, cs[:n_main])
        total[r0 + 512:r0 + 1024] += cs[n_main:n_main + D0CS]
        # d4-mirror: pair core's rows 512:1024 get their d4 cols 0:512 mass
        gidx4 = (r0 + POS_Q0 + NCOL + np.arange(D4CS)) % TWO_N
        np.add.at(total, gidx4, cs[n_main + D0CS:])
    denom = total - np.exp(TEMP_SCALE)
    terms = np.log(denom) - np.log(posg)
    return float(terms.mean())


def kernel(z_i, z_j):
    nc = _get_nc()
    in_maps = _prepare_in_maps(z_i, z_j)
    res = run_bass_kernel_spmd(nc, in_maps, core_ids=list(range(N_CORES)))
    return np.float32(_combine(res.results))


if __name__ == "__main__":
    rng = np.random.default_rng(0)
    z_i = rng.standard_normal((4096, 256), dtype=np.float32)
    z_j = rng.standard_normal((4096, 256), dtype=np.float32)
    print("loss:", kernel(z_i, z_j))



# revision 71
# speedup vs baseline: 1.0440x; 1.0440x over previous
"""NT-Xent loss on 8 Trainium2 NeuronCores — fp8 + symmetric 5/8 scheme.

Math: z = concat(z_i, z_j) [8192, 256]; E = exp(2*cos_sim(z)) is
symmetric, so row sums = column sums. Blocked over 8 superblocks of
1024 rows, core c owns rows [1024c, 1024c+1024) and computes sim
against column superblocks d = 0..4 (its own + the next four, mod 8).
d in {1,2,3} blocks are mirrored via COLUMN sums (row sums of the
uncomputed transpose block). The d=4 block forms a transpose-dual pair
across cores (c, c+4): rows m>=4 compute only cols [512:1024) and the
missing cols [0:512) mass is mirrored from the pair core's colsum over
its m0..3 rows of cols [512:1024) (d4-mirror; only the two diagonal
512-squares are double-computed, counted once each). The host
assembles denom = sum - e^2, takes ln in fp64, and averages — O(2N).

Per core (identical SPMD program on column-rotated inputs):
  - Host normalizes rows in fp64, scales by 16, quantizes to fp8 e4m3,
    ships chunk-major znt[p, 2c+h, j] = q[1024c+j, 128h+p] so every
    1024-col superblock is 2 KiB contiguous per partition on both DMA
    sides. Two DMA rings in baseline order (sync: c0,c2,c4; gpsimd:
    c1,id,c3); ones comes from a gpsimd memset (a [P,1] DMA is 128
    tiny descriptors that stall a ring). A third ring (scalar HWDGE)
    and finer chunk splits measured WORSE: all rings share one AXI
    port (~200 GB/s) and late-deadline chunks steal bandwidth from
    the critical first window's data.
  - Sim matmuls: fp8 DoubleRow, one matmul per [128, 512] output
    covering the full K=256 contraction. PSUM = 256*sim.
  - d0-triangle skip: for row-chunks m>=4, cols [0:512) of the self
    superblock are strictly below the diagonal — skipped, mass comes
    back via the d0-mirror colsum (rows 0:512 of cols [512:1024)).
  - ACT exps [128, 2048] PSUM windows (scale 1/128) with fused row-sum
    accumulation, writing exp as bf16 to SBUF (esb) for the colsum
    path. FIVE of the 16 q-windows are offloaded to DVE via int16
    Schraudolph: int16(A/2^16*x + B/2^16) IS the bf16 bit pattern of
    exp(x), written straight into esb via .bitcast(I16); the rowsum is
    a flat bf16 copy-reduce (CACHE_REDUCE has no 2x uop; fold trees
    measured worse under DVE<->gpsimd SBUF contention).
  - A d=4 window rides behind the window FOLLOWING each Sch window:
    the 2-slot PSUM rotation goes sch(A) -> next-q(B) -> ride(A, freed
    by pass1) -> q+2(B). Riding directly behind the Sch window puts
    the ride in slot B and costs ~2 us of ACT idle per Sch cycle
    (pass1 holds slot A ~2.3 us). Trailing d4 windows m5+m6 share one
    PSUM tile (both 512-col after d4-mirror), m7 takes the next slot.
  - Colsum trees (bf16 pair-adds over the 8 m-chunks, emitted
    incrementally as chunks land): q0 + d0 trees on GpSimd (deep DVE
    queues around the Sch offloads cost more than gpsimd<->DVE SBUF
    contention), q1 + d4sum trees on DVE (2x, tight tail deadline).
    Tail colsum ones-matmuls grouped by READINESS: (0,1,6,7) = esum0/
    d0sum/d4sum are ready mid-stream; (2,3,4,5) = esum1 waits only on
    the q1 tree (~2.4 us after the last window).
  - Outputs: rowpos [128, 16] (rowsum partials | exp'd positives) and
    colsums [1, 4096] — ONE DMA each; every DMA_DIRECT2D costs ~0.6 us
    of engine issue time, so per-block output DMAs wreck the tail.

Measured pitfalls baked in (each cost a hardware run to learn):
  - DVE tensor_reduce AND tensor_scalar CACHE_REDUCE have no 2x uop.
  - gpsimd partition_all_reduce is ~5x slower than its cost model
    (use ones-matmuls); gpsimd tensor_tensor bf16 adds DO match the
    model (~2.1 us per [128,1024]) and gpsimd has no PSUM port; its
    scalar_tensor_tensor with accum_out does not compile (neuronxcc).
  - DVE<->gpsimd SBUF contention stretches overlapping ops 1.5-2.5x
    (ACT never stretches). Fewer, larger ops win.
  - fp8 DoubleRow streams ~0.5-0.6 ns/moving-row; ACT ~0.96 ns/col
    with no 2x for any dtype; ACT accum flush = 284 ns per op.
  - Framework preamble ~7 us, DMA doorbell ~1.6 us after issue, final
    barrier+drain ~4.5 us: ~13 us of fixed overhead.
  - Window splitting and ride-spreading LOSE (3 PSUM consumers per
    2-slot turn starves the rotation: +6..+9 us). q1-before-q0 LOSES
    (window 0 then needs 768 KiB of DMA instead of 512 KiB).
  - Run-to-run jitter is +-2-3 us, and the device occasionally runs
    whole kernels at ~9% lower clock (check: 2048-col ACTIVATE =
    1967 ns at full clock) — verify clock before comparing runs.

fp8 error budget: e4m3 sigma~3.6%/elem -> sim noise sigma~3e-3 which
averages out in row sums and the 8192-row mean; Schraudolph adds ~3%
per-element on 5/16 of the denominator mass -> ~1e-4 loss error.
The q1 tree stops at the half-sums t1_1_0/t1_1_4; the tail colsum
matmuls accumulate the two halves per 512-col PSUM bank (a matmul may
not cross a bank boundary), so the tail chain is only
(1,7)exp -> t1_0_6 -> t1_1_4 -> 4 stop-matmuls -> copies -> one DMA.
Verified: 61356-62688 ns HW (baseline 63154 same-day), rel err 1.2e-4.
"""

import os
import sys

sys.path.insert(0, "/opt/trn_rl_repo")
os.environ.setdefault("MYCRO_LOCAL_CACHE", "1")

import numpy as np

import concourse.bass as bass
import concourse.mybir as mybir
from concourse import bacc, tile
from concourse.bass_utils import run_bass_kernel_spmd

F32 = mybir.dt.float32
BF16 = mybir.dt.bfloat16
FP8 = mybir.dt.float8e4
I32 = mybir.dt.int32
I16 = mybir.dt.int16
AF = mybir.ActivationFunctionType
ALU = mybir.AluOpType
DROW = mybir.MatmulPerfMode.DoubleRow

N_CORES = 8
TWO_N = 8192
D = 256
P = 128
ROWS_PER_CORE = TWO_N // N_CORES  # 1024
M_CHUNKS = ROWS_PER_CORE // P     # 8 local row chunks
NCOL = 512                        # matmul free dim (one PSUM bank)
QCOL = 2048                       # ACT window for q=0,1
N_CHUNKS = 5                      # 1024-col superblocks held per core
LCOLS = 5 * ROWS_PER_CORE         # 5120 local cols (superblocks d=0..4)
CS_LO, CS_HI = 1024, 4096         # colsum region (d=1..3)
D0CS = 512                        # d0 below-diag mirror segment (cols
                                  # 512:1024 summed over rows 0:512)
D4CS = 512                        # d4-mirror segment (cols 4608:5120
                                  # summed over rows 0:512)
CS_TOT = CS_HI - CS_LO + D0CS + D4CS   # 4096 colsum outputs
POS_Q0 = 4096                     # start of the d=4 window
N_WIN = 3                         # rowsum slots per m: q0, q1, d4
TEMP_SCALE = 2.0
QSCALE = 16.0                     # host fp8 quant scale; psum = 256*sim
ACT_SCALE = TEMP_SCALE / (QSCALE * QSCALE)   # exp(psum/128)
POS_SCALE = TEMP_SCALE / (QSCALE * QSCALE)

# Schraudolph: exp(x) ~ bits_as_f32(int32(2^23/ln2 * x + 127*2^23 - C))
SCH_A = ACT_SCALE * (1 << 23) / float(np.log(2.0))   # applied to psum
SCH_B = float(127 * (1 << 23) - 366393)
# int16 variant: the int16 of (SCH_A/2^16 * x + SCH_B/2^16) is the upper
# half of the f32 bit pattern = the bf16 of exp(x), written directly
SCH_A16 = SCH_A / 65536.0
SCH_B16 = SCH_B / 65536.0
# windows (q, m) whose exp+rowsum runs on DVE instead of ACT
SCH_WINDOWS = {(0, 2), (0, 5), (1, 0), (1, 3), (1, 6)}
# Sch windows whose pass2 rowsum-reduce runs on ACT instead of DVE.
# Tried {(1,0)}: +2.3 us — DVE is ~92% busy mid-stream vs ACT ~78%,
# but the 2.25 us ACT reduce overflows ACT's ~1.3 us bubbles and
# delays the window chain. Keep empty.
SCH_ACT_REDUCE = set()

_NC_CACHE = {}


def _build_nc():
    nc = bacc.Bacc(
        "TRN2",
        target_bir_lowering=False,
        debug=False,
        enable_asserts=False,
        num_devices=N_CORES,
    )
    # chunk-major layout: znt[p, 2*c + h, j] = q[1024*c + j, 128*h + p]
    # so each 1024-col superblock is 2 KiB contiguous per partition on
    # both the DRAM and SBUF side (fast DMA lines)
    znt = nc.dram_tensor("znt", [P, 2 * N_CHUNKS, 1024], FP8,
                         kind="ExternalInput")
    ident = nc.dram_tensor("ident", [P, P], BF16, kind="ExternalInput")
    rowpos_d = nc.dram_tensor("rowpos", [P, 2 * M_CHUNKS], F32,
                              kind="ExternalOutput")
    colsums_d = nc.dram_tensor("colsums", [1, CS_TOT], F32,
                               kind="ExternalOutput")

    with tile.TileContext(nc) as tc:
        with (
            tc.tile_pool(name="big", bufs=1) as big,
            tc.tile_pool(name="esbp", bufs=1) as esbp,
            tc.tile_pool(name="work", bufs=2) as work,
        ):
            znt_sb = big.tile([P, 2 * N_CHUNKS, 1024], FP8)

            def zv(c0, width=NCOL):
                """[128, 2, width] matmul-operand view of global cols
                [c0, c0+width) — must stay inside one 1024-col chunk."""
                c, j = divmod(c0, 1024)
                assert j + width <= 1024
                return znt_sb[:, 2 * c:2 * c + 2, j:j + width]

            # two DMA rings, exactly the baseline's queue assignment and
            # order (a third ring and finer splits measured WORSE: the
            # rings share one AXI port and competition from late-deadline
            # chunks slows the critical first chunks). Chunk-major layout
            # still gives 2 KiB contiguous lines on both sides.
            # c1 is split across both rings so window (0,0)'s full data
            # (c0 + c1) lands ~1 us sooner; c1b heads the gpsimd ring
            nc.sync.dma_start(znt_sb[:, 0:2, :], znt[:, 0:2, :])
            nc.gpsimd.dma_start(znt_sb[:, 2:4, 512:1024],
                                znt[:, 2:4, 512:1024])
            id_sb = big.tile([P, P], BF16)
            ones_sb = big.tile([P, 1], BF16)
            nc.gpsimd.memset(ones_sb[:], 1.0)
            nc.sync.dma_start(znt_sb[:, 2:4, 0:512], znt[:, 2:4, 0:512])
            nc.gpsimd.dma_start(id_sb[:], ident[:])
            nc.sync.dma_start(znt_sb[:, 4:6, :], znt[:, 4:6, :])
            nc.gpsimd.dma_start(znt_sb[:, 6:8, :], znt[:, 6:8, :])
            nc.sync.dma_start(znt_sb[:, 8:10, :], znt[:, 8:10, :])

            # preload the Exp activation table while DMAs stream; read a
            # memset scratch so the preload has no DMA dependency
            tblin = big.tile([P, 1], F32)
            nc.gpsimd.memset(tblin[:], 0.0)
            tbl = big.tile([P, 1], F32)
            nc.scalar.activation(tbl[:], tblin[:], AF.Exp)

            sums = big.tile([P, M_CHUNKS * N_WIN], F32)
            # rowpos[:, 0:8] = rowsum partials (stot), [:, 8:16] = posv;
            # combined so one DMA issue flushes both
            rowpos = big.tile([P, 2 * M_CHUNKS], F32)
            pos = rowpos[:, M_CHUNKS:2 * M_CHUNKS]
            # exp windows for q=0,1 (cols 0:4096), bf16, m-major
            esb = [esbp.tile([P, M_CHUNKS, QCOL], BF16, tag=f"esb{q}",
                             name=f"esb{q}")
                   for q in range(2)]
            esum = [esbp.tile([P, 1024], BF16, tag="esum0", name="esum0"),
                    esbp.tile([P, QCOL], BF16, tag="esum1", name="esum1")]
            colsum_sb = big.tile([1, CS_TOT], F32)
            d4buf = esbp.tile([P, M_CHUNKS, 1024], BF16, tag="d4b",
                              name="d4buf")

            tree_t = {}

            def tree_step(dst, src_m, sl, width, q, m):
                """Emit the bf16 adds that become ready once chunk m of
                src_m[:, :, sl] is written. q0's tree finishes with a
                full m-sum in dst (gpsimd: deep DVE queues around the
                Schraudolph offloads cost more than the gpsimd<->DVE
                SBUF contention they avoid). q1's tree STOPS at the two
                half-sums t1_1_0/t1_1_4 — the colsum matmuls accumulate
                them in PSUM, cutting the final 1.2 us add off the
                tail-gating chain. (Tried q1's early half on gpsimd
                too — its slow 2048-wide adds stretch DVE via SBUF
                contention more than they relieve its queue: +1.5 us.)"""
                eng = nc.gpsimd if q == 0 else nc.vector
                def t(tag):
                    if tag not in tree_t:
                        tree_t[tag] = work.tile([P, width], BF16, tag=tag,
                                                bufs=1, name=tag)
                    return tree_t[tag]
                if m % 2 == 1:
                    a = t(f"t{q}_0_{m - 1}")
                    eng.tensor_tensor(a[:], src_m[:, m - 1, sl],
                                      src_m[:, m, sl], ALU.add)
                if m == 3:
                    b = t(f"t{q}_1_0")
                    eng.tensor_tensor(b[:], t(f"t{q}_0_0")[:],
                                      t(f"t{q}_0_2")[:], ALU.add)
                if m == 7:
                    b = t(f"t{q}_1_4")
                    eng.tensor_tensor(b[:], t(f"t{q}_0_4")[:],
                                      t(f"t{q}_0_6")[:], ALU.add)
                    if q == 0:
                        eng.tensor_tensor(dst, t(f"t{q}_1_0")[:],
                                          b[:], ALU.add)

            with tc.tile_pool(name="psum", bufs=2, space="PSUM") as psum_pool:
                def d4_window(m, ptf=None, base=0):
                    """d=4 window interleaved into the q stream: MMs into
                    a half-used sim-tag tile, exp to SBUF d4buf, post-exp
                    diagonal extract (2x bf16 STT, no PSUM interlock).
                    d4-mirror: the (c, c+4) block pair is transpose-dual,
                    so rows m>=4 compute only cols [512:1024) and recover
                    cols [0:512) mass from the PAIR core's colsum over its
                    m0..3 rows of cols [512:1024) (the d4sum tree below);
                    only the two diagonal 512-squares are double-computed,
                    and those enter rowsums once each (no double count).
                    Pass ptf/base to pack two 512-col windows in one tile
                    (the trailing windows otherwise serialize on slot
                    rotation)."""
                    if ptf is None:
                        ptf = psum_pool.tile([P, QCOL], F32, tag="sim",
                                             name="ptd4")
                    pt = ptf[:, base:base + 1024]
                    lhsT = zv(m * P, P)
                    lo4 = 0 if m < 4 else NCOL
                    for nn in range(lo4 // NCOL, 2):
                        col = POS_Q0 + nn * NCOL
                        nc.tensor.matmul(
                            pt[:, nn * NCOL:(nn + 1) * NCOL],
                            lhsT,
                            zv(col),
                            start=True, stop=True, perf_mode=DROW)
                    col_ix = m * N_WIN + 2
                    nc.scalar.activation(
                        d4buf[:, m, lo4:], pt[:, lo4:], AF.Exp,
                        scale=ACT_SCALE,
                        accum_out=sums[:, col_ix:col_ix + 1])
                    off = m * P
                    scr = work.tile([P, P], BF16, tag="extr")
                    nc.vector.scalar_tensor_tensor(
                        out=scr[:], in0=d4buf[:, m, off:off + P],
                        scalar=1.0, in1=id_sb[:],
                        op0=ALU.mult, op1=ALU.mult,
                        accum_out=pos[:, m:m + 1])
                    if m == 3:
                        # d4-mirror tree on DVE (tried gpsimd: +2.5 us
                        # at full clock — even these narrow 512-col adds
                        # land mid-q1-phase and stretch the DVE pacer
                        # via SBUF contention; gpsimd only breaks even
                        # for work far from DVE-busy regions)
                        ta = work.tile([P, NCOL], BF16, tag="d4a",
                                       bufs=1, name="d4a")
                        nc.vector.tensor_tensor(
                            ta[:], d4buf[:, 0, 512:1024],
                            d4buf[:, 1, 512:1024], ALU.add)
                        tb = work.tile([P, NCOL], BF16, tag="d4b2",
                                       bufs=1, name="d4b2")
                        nc.vector.tensor_tensor(
                            tb[:], d4buf[:, 2, 512:1024],
                            d4buf[:, 3, 512:1024], ALU.add)
                        td = work.tile([P, NCOL], BF16, tag="d4s",
                                       bufs=1, name="d4sum")
                        nc.vector.tensor_tensor(td[:], ta[:], tb[:],
                                                ALU.add)
                        tree_t["d4sum"] = td

                # q0 phase first (q1-first measured far worse: window 0
                # then needs c0+c2+c3 = 768 KiB before it can finish vs
                # 512 KiB for q0-first). A d4 window rides behind the
                # window FOLLOWING each DVE-offloaded one: the 2-slot
                # rotation then goes sch(A) -> next-q(B) -> ride(A,
                # freed by pass1) -> q+2(B). Riding directly behind the
                # Sch window put the ride in slot B and made the next
                # q-window wait out pass1's ~2.3 us hold of slot A
                # (~2 us ACT gap per Sch cycle).
                d4_done = 0
                ride_pending = False
                for q in (0, 1):        # esb cols: q0 [0:2048), q1 [2048:4096)
                    for m in range(M_CHUNKS):
                        pt = psum_pool.tile([P, QCOL], F32, tag="sim")
                        lhsT = zv(m * P, P)
                        # cols [0:512) of the self-superblock are strictly
                        # below the diagonal for rows m>=4: skip them and
                        # recover the row-sum mass from the d0-mirror
                        # colsum over rows m0..3 of cols [512:1024)
                        lo = NCOL if (q == 0 and m >= 4) else 0
                        for nn in range(lo // NCOL, QCOL // NCOL):
                            col = q * QCOL + nn * NCOL
                            nc.tensor.matmul(
                                pt[:, nn * NCOL:(nn + 1) * NCOL],
                                lhsT,
                                zv(col),
                                start=True, stop=True, perf_mode=DROW)
                        col_ix = m * N_WIN + q
                        if (q, m) in SCH_WINDOWS:
                            # Schraudolph exp on DVE, single pass: int16 of
                            # (A16*x + B16) IS the bf16 of exp(x), written
                            # straight into esb (frees the PSUM bank);
                            # rowsum via a flat bf16 copy-reduce (folds and
                            # multi-op variants measured WORSE under
                            # DVE<->gpsimd SBUF contention)
                            nc.vector.tensor_scalar(
                                esb[q][:, m, lo:].bitcast(I16),
                                pt[:, lo:], SCH_A16, SCH_B16,
                                ALU.mult, ALU.add)
                            scr = work.tile([P, QCOL], BF16, tag="sch",
                                            name="schscr")
                            # NOTE: tensor_tensor_reduce(max(e,e)) as a
                            # 2x-mode shot passed CoreSim but crashed at
                            # HW execution (INTERNAL runtime error) —
                            # stay on the 1x tensor_scalar cache-reduce
                            nc.vector.tensor_scalar(
                                scr[:, lo:], esb[q][:, m, lo:],
                                1.0, 0.0, ALU.mult, ALU.add,
                                accum_out=sums[:, col_ix:col_ix + 1])
                        else:
                            nc.scalar.activation(
                                esb[q][:, m, lo:], pt[:, lo:], AF.Exp,
                                scale=ACT_SCALE,
                                accum_out=sums[:, col_ix:col_ix + 1])
                        if (q, m) == (0, 3):
                            # d0-mirror tree: sum rows m0..3 of cols
                            # [512:1024) (bf16, 2x mode)
                            ta = work.tile([P, D0CS], BF16, tag="d0a",
                                           bufs=1, name="d0a")
                            nc.gpsimd.tensor_tensor(
                                ta[:], esb[0][:, 0, 512:1024],
                                esb[0][:, 1, 512:1024], ALU.add)
                            tb = work.tile([P, D0CS], BF16, tag="d0b",
                                           bufs=1, name="d0b")
                            nc.gpsimd.tensor_tensor(
                                tb[:], esb[0][:, 2, 512:1024],
                                esb[0][:, 3, 512:1024], ALU.add)
                            td = work.tile([P, D0CS], BF16, tag="d0s",
                                           bufs=1, name="d0sum")
                            nc.gpsimd.tensor_tensor(td[:], ta[:], tb[:],
                                                    ALU.add)
                            tree_t["d0sum"] = td
                        # colsum m-tree, incrementally as chunks land
                        if q == 0:
                            tree_step(esum[0][:], esb[0], slice(1024, 2048),
                                      1024, 0, m)
                        else:
                            tree_step(esum[1][:], esb[1], slice(0, QCOL),
                                      QCOL, 1, m)
                        if ride_pending:
                            d4_window(d4_done)
                            d4_done += 1
                            ride_pending = False
                        if (q, m) in SCH_WINDOWS and d4_done < 5:
                            ride_pending = True
                # trailing d4 windows: m5+m6 share one PSUM tile (both
                # are 512-col), m7 takes the next rotation slot
                assert d4_done == 5
                pair = psum_pool.tile([P, QCOL], F32, tag="sim",
                                      name="ptd4pair")
                d4_window(5, ptf=pair, base=0)
                d4_window(6, ptf=pair, base=1024)
                d4_window(7)

                # rowsum partials + posv go out as soon as the last
                # window's accum lands (before the colsum tail); one
                # combined DMA (each DMA_DIRECT2D costs ~0.6 us of
                # sync-queue issue time)
                nc.vector.tensor_reduce(
                    rowpos[:, 0:M_CHUNKS],
                    sums[:].rearrange("p (m q) -> p m q", q=N_WIN),
                    axis=mybir.AxisListType.X,
                    op=ALU.add,
                )
                nc.sync.dma_start(rowpos_d[:], rowpos[:])

                # colsum matmuls ride the tail of the sim ring, grouped
                # by READINESS. Group A (esum0 + d0sum + d4sum, blocks
                # 0,1,6,7) is complete by mid-stream and overlaps the
                # last windows. Group B (esum1 region, blocks 2..5) is
                # TWO wide [1, 2048] ones-matmuls accumulating the q1
                # half-tree sums in PSUM: the t1_1_0 matmul is ready
                # mid-stream, so the tail chain is only
                # (1,7)exp -> t1_0_6 -> t1_1_4 -> one matmul -> copies.
                ptcsA = psum_pool.tile([P, QCOL], F32, tag="sim",
                                       name="ptcsA")
                for i, b in enumerate((0, 1, 6, 7)):
                    if b == 6:
                        src = tree_t["d0sum"][:]
                    elif b == 7:
                        src = tree_t["d4sum"][:]
                    else:
                        src = esum[0][:, b * NCOL:(b + 1) * NCOL]
                    nc.tensor.matmul(ptcsA[0:1, i * NCOL:(i + 1) * NCOL],
                                     ones_sb[:], src,
                                     start=True, stop=True)
                for half in range(2):
                    b0 = (0, 6)[half]
                    psl = ptcsA[0:1, 2 * half * NCOL:2 * (half + 1) * NCOL]
                    dst = colsum_sb[:, b0 * NCOL:(b0 + 2) * NCOL]
                    if half == 0:
                        nc.vector.tensor_copy(dst, psl)
                    else:
                        nc.scalar.copy(dst, psl)

                ptcsB = psum_pool.tile([P, QCOL], F32, tag="sim",
                                       name="ptcsB")
                # per-bank accumulating pairs (a matmul may not cross a
                # PSUM bank): the 4 t1_1_0 matmuls are ready mid-stream,
                # only the 4 t1_1_4 ones ride the tail chain
                for i in range(4):
                    sl = slice(i * NCOL, (i + 1) * NCOL)
                    nc.tensor.matmul(ptcsB[0:1, sl], ones_sb[:],
                                     tree_t["t1_1_0"][:, sl],
                                     start=True, stop=False)
                for i in range(4):
                    sl = slice(i * NCOL, (i + 1) * NCOL)
                    nc.tensor.matmul(ptcsB[0:1, sl], ones_sb[:],
                                     tree_t["t1_1_4"][:, sl],
                                     start=False, stop=True)
                nc.vector.tensor_copy(colsum_sb[:, 1024:2048],
                                      ptcsB[0:1, 0:1024])
                nc.scalar.copy(colsum_sb[:, 2048:3072],
                               ptcsB[0:1, 1024:2048])
                nc.sync.dma_start(colsums_d[:], colsum_sb[:])

    nc.compile()
    return nc


def _get_nc():
    if "nc" not in _NC_CACHE:
        _NC_CACHE["nc"] = _build_nc()
    return _NC_CACHE["nc"]


def _prepare_in_maps(z_i, z_j):
    import ml_dtypes

    z = np.concatenate(
        [np.asarray(z_i, np.float64), np.asarray(z_j, np.float64)], axis=0
    )
    zn = z / np.linalg.norm(z, axis=1, keepdims=True)
    q = (zn * QSCALE).astype(np.float32).astype(ml_dtypes.float8_e4m3)
    # znt[p, h, j] = q[j, h*128 + p]
    znt = np.ascontiguousarray(q.T.reshape(2, P, TWO_N).transpose(1, 0, 2))
    ident = np.eye(P, dtype=ml_dtypes.bfloat16)
    in_maps = []
    for c in range(N_CORES):
        zc = np.roll(znt, -ROWS_PER_CORE * c, axis=2)[:, :, :LCOLS]
        # chunk-major: znt10[p, 2*c + h, j] = zc[p, h, 1024*c + j]
        zc10 = zc.reshape(P, 2, N_CHUNKS, 1024).transpose(0, 2, 1, 3)
        zc10 = np.ascontiguousarray(zc10.reshape(P, 2 * N_CHUNKS, 1024))
        in_maps.append(
            {"znt": zc10, "ident": ident})
    return in_maps


def _combine(results):
    """Assemble the loss from per-core rowsum/colsum/pos partials."""
    total = np.zeros(TWO_N, dtype=np.float64)
    posg = np.zeros(TWO_N, dtype=np.float64)
    for c in range(N_CORES):
        r0 = c * ROWS_PER_CORE
        rp = np.asarray(results[c]["rowpos"], np.float64)   # [128, 16]
        rs, pv = rp[:, :M_CHUNKS], rp[:, M_CHUNKS:]
        for m in range(M_CHUNKS):
            gsl = slice(r0 + m * P, r0 + (m + 1) * P)
            total[gsl] += rs[:, m]
            posg[gsl] = pv[:, m]
        cs = np.asarray(results[c]["colsums"], np.float64).ravel()  # [4096]
        n_main = CS_HI - CS_LO
        gidx = (r0 + CS_LO + np.arange(n_main)) % TWO_N
        np.add.at(total, gidx, cs[:n_main])
        total[r0 + 512:r0 + 1024] += cs[n_main:n_main + D0CS]
        # d4-mirror: pair core's rows 512:1024 get their d4 cols 0:512 mass
        gidx4 = (r0 + POS_Q0 + NCOL + np.arange(D4CS)) % TWO_N
        np.add.at(total, gidx4, cs[n_main + D0CS:])
    denom = total - np.exp(TEMP_SCALE)
    terms = np.log(denom) - np.log(posg)
    return float(terms.mean())


def kernel(z_i, z_j):
    nc = _get_nc()
    in_maps = _prepare_in_maps(z_i, z_j)
    res = run_bass_kernel_spmd(nc, in_maps, core_ids=list(range(N_CORES)))
    return np.float32(_combine(res.results))


if __name__ == "__main__":
    rng = np.random.default_rng(0)
    z_i = rng.standard_normal((4096, 256), dtype=np.float32)
    z_j = rng.standard_normal((4096, 256), dtype=np.float32)
    print("loss:", kernel(z_i, z_j))

